# revision 25
# baseline (speedup 1.0000x reference)
"""GAT (3x GATConv + pool + dense tail) on 8 trn2 NeuronCores.

v2 design:
- Nodes padded per graph to 128-multiples; tiles assigned contiguously to
  cores; per-core tile processing order sorted by descending edge count so
  the SPMD-shared per-step gather chunk counts (max over cores) stay tight.
- Gather tables in bf16, rows [h | 1 | s1 | pad] of 128/256/384 elems.
- Each layer's table is produced by the PREVIOUS layer's phase-B epilogue
  (fused next-layer projection) on the owning core, then AllGather'd to all
  cores' DRAM.  Layer-1's table comes from a small sharded phase A over the
  core's own x columns.
- Phase B per tile: dma_gather edge blocks; per block two fused DVE ops
  (s2e via scalar_tensor_tensor is_equal*S2B with accumulate; sel via
  tensor_scalar is_equal*ex) and one accumulate matmul.
- Pooling: sum via N=1 matmuls into PSUM columns, max via per-partition
  pad offsets + PE transpose + PSUM-direct reduces.
- Dense tail fp32 in [feat, G] layout, replicated on all cores.
"""

import os
import numpy as np
import ml_dtypes

import concourse.bass as bass
import concourse.bacc as bacc
import concourse.mybir as mybir
import concourse.tile as tile
from concourse._compat import cdiv
from concourse.masks import make_identity
from concourse.tile_rust import add_dep_helper

P = 128
NCORES = 8
NEG_SLOPE = 0.2
NEG_BIG = -1.0e30
CALL_CHUNKS = int(os.environ.get("GAT_CC", "5"))
MAXSLOT = 12
WBATCH = 4             # stage-write batching (tiles per DMA)

F32 = mybir.dt.float32
BF16 = mybir.dt.bfloat16
I32 = mybir.dt.int32
I16 = mybir.dt.int16

NPBF16 = ml_dtypes.bfloat16

TAIL_SHAPES = dict(d1=(512, 256), d2=(256, 128), d3=(128, 64), mean=(256, 64),
                   d4=(64, 64), d5=(64, 128), d6=(128, 256), d7=(256, 128))


def row_elems(do):
    """bf16 elements per table row (256B-multiple)."""
    return int(cdiv(do + 2, 128) * 128)


def _wrap_idx(flat):
    n = len(flat)
    assert n % 16 == 0
    w = np.zeros((P, n // 16), np.int16)
    k = np.arange(n)
    w[(k % 16)[None, :] + 16 * np.arange(8)[:, None], (k // 16)[None, :]] = \
        np.asarray(flat, np.int16)[None, :]
    return w


def preprocess(x, edge_index, batch, G, layer_dims):
    N, F_IN = x.shape
    src = np.asarray(edge_index[0]).astype(np.int64)
    dst = np.asarray(edge_index[1]).astype(np.int64)
    batch = np.asarray(batch).astype(np.int64)

    cnt = np.bincount(batch, minlength=G).astype(np.int64)
    blk = np.maximum(1, np.ceil(cnt / P).astype(np.int64)) * P
    gstart_pad = np.concatenate([[0], np.cumsum(blk)])
    T = int(cdiv(gstart_pad[-1] // P, NCORES) * NCORES)
    TOWN = T // NCORES
    SH = (TOWN + 1) * P                     # shard rows (own tiles + pad tile)
    NTT = NCORES * SH
    assert NTT <= 65536, NTT
    VIEWB = NTT - 32768                     # start row of table view B
    PADLO = TOWN * P                        # core 0's pad tile (row < 32768)
    PADHI = (NCORES - 1) * SH + TOWN * P    # core 7's pad tile (row >= VIEWB)
    assert PADLO < 32768 and PADHI >= VIEWB

    gstart_real = np.concatenate([[0], np.cumsum(cnt)])
    node_pos = np.zeros(N, np.int64)        # graph-padded position
    for g in range(G):
        a, b = gstart_real[g], gstart_real[g + 1]
        node_pos[a:b] = gstart_pad[g] + np.arange(b - a)

    # edges incl. self-loops, in padded-position space
    sl = np.arange(N)
    srcpos = node_pos[np.concatenate([src, sl])]
    dstpos = node_pos[np.concatenate([dst, sl])]
    src_tile = srcpos // P
    dst_tile = dstpos // P

    # per-core processing order: own tiles sorted by descending edge count
    tile_edge_cnt = np.bincount(dst_tile, minlength=T)
    perm = np.zeros((NCORES, TOWN), np.int64)     # step -> local tile
    step_of = np.zeros((NCORES, TOWN), np.int64)  # local tile -> step
    for c in range(NCORES):
        cnts = tile_edge_cnt[c * TOWN:(c + 1) * TOWN]
        order = np.argsort(-cnts, kind="stable")
        perm[c] = order
        step_of[c, order] = np.arange(TOWN)

    def pos_to_row(pos):
        t = pos // P
        c = t // TOWN
        s = step_of[c, t % TOWN]
        return c * SH + s * P + pos % P

    src_row = pos_to_row(srcpos)
    dst_row = pos_to_row(dstpos)
    dst_core = dst_tile // TOWN
    dst_step = step_of[dst_core, dst_tile % TOWN]
    dst_loc = dstpos % P

    # group edges by (core, step, half)
    lo_mask = src_row < 32768
    K_lo = np.zeros(TOWN, np.int64)
    K_hi = np.zeros(TOWN, np.int64)
    per_cs = {}
    for c in range(NCORES):
        m_c = dst_core == c
        for s in range(TOWN):
            m = m_c & (dst_step == s)
            ml = m & lo_mask
            mh = m & ~lo_mask
            per_cs[(c, s)] = (src_row[ml], dst_loc[ml], src_row[mh], dst_loc[mh])
            K_lo[s] = max(K_lo[s], cdiv(ml.sum(), P))
            K_hi[s] = max(K_hi[s], cdiv(mh.sum(), P))
    for s in range(TOWN):
        if K_lo[s] + K_hi[s] == 0:
            K_lo[s] = 1
    nchunks = int((K_lo + K_hi).sum())

    idx16 = np.zeros((NCORES, P, nchunks * 8), np.int16)
    dstloc = np.zeros((NCORES, P, nchunks), np.float32)
    for c in range(NCORES):
        ch = 0
        for s in range(TOWN):
            s_lo, d_lo, s_hi, d_hi = per_cs[(c, s)]
            for (s_arr, d_arr, K, is_lo) in ((s_lo, d_lo, int(K_lo[s]), True),
                                             (s_hi, d_hi, int(K_hi[s]), False)):
                if K == 0:
                    continue
                ns = K * P
                if is_lo:
                    vidx = np.full(ns, PADLO, np.int64)
                    vidx[: len(s_arr)] = s_arr
                else:
                    vidx = np.full(ns, PADHI - VIEWB, np.int64)
                    vidx[: len(s_arr)] = s_arr - VIEWB
                assert vidx.min() >= 0 and vidx.max() < 32768
                dl = np.zeros(ns, np.int64)
                dl[: len(d_arr)] = d_arr
                idx16[c, :, ch * 8:(ch + K) * 8] = _wrap_idx(vidx)
                dstloc[c, :, ch:ch + K] = dl.reshape(K, P).T.astype(np.float32)
                ch += K
        assert ch == nchunks

    # per-core x slice in processing order: [F_IN, TOWN*P]
    xcore = np.zeros((NCORES, F_IN, TOWN * P), np.float32)
    xT = np.asarray(x, np.float32).T            # [F_IN, N]
    all_rows = pos_to_row(node_pos)             # per real node
    for c in range(NCORES):
        m = (all_rows >= c * SH) & (all_rows < c * SH + TOWN * P)
        xcore[c][:, all_rows[m] - c * SH] = xT[:, m]

    # pad masks per core in processing order: [P, TOWN]
    padT = np.zeros((NCORES, P, TOWN), np.float32)
    for c in range(NCORES):
        rr = all_rows[(all_rows >= c * SH) & (all_rows < c * SH + TOWN * P)] - c * SH
        padT[c][rr % P, rr // P] = 1.0

    # graph slots per core (over steps)
    tile_graph = np.searchsorted(gstart_pad, np.arange(T) * P, side="right") - 1
    tile_graph = np.minimum(tile_graph, G - 1)
    slot_graph = np.full((NCORES, MAXSLOT), -1, np.int64)
    slot_mask = np.zeros((NCORES, MAXSLOT, TOWN), np.float32)
    for c in range(NCORES):
        gs = []
        for s in range(TOWN):
            g = int(tile_graph[c * TOWN + perm[c][s]])
            if g not in gs:
                gs.append(g)
            slot_mask[c, gs.index(g), s] = 1.0
        assert len(gs) <= MAXSLOT, len(gs)
        slot_graph[c, :len(gs)] = gs

    gsrc = []
    for g in range(G):
        locs = [(c, sl) for c in range(NCORES) for sl in range(MAXSLOT)
                if slot_graph[c, sl] == g]
        assert 1 <= len(locs) <= 2, (g, locs)
        if len(locs) == 1:
            locs = locs * 2
        gsrc.append(locs)

    recip_cnt = (1.0 / np.maximum(cnt, 1.0)).astype(np.float32)

    return dict(N=N, F_IN=F_IN, G=G, T=T, TOWN=TOWN, SH=SH, NTT=NTT,
                VIEWB=VIEWB, layer_dims=layer_dims,
                K_lo=K_lo, K_hi=K_hi, nchunks=nchunks, idx16=idx16,
                dstloc=dstloc, xcore=xcore, padT=padT,
                slot_mask=slot_mask, gsrc=gsrc, recip_cnt=recip_cnt)


def make_inputs(pp, weights):
    layer_dims = pp["layer_dims"]
    ins_shared = {}
    for li, (di, do) in enumerate(layer_dims):
        W = np.asarray(weights[f"W{li+1}"], np.float32)
        a_s = np.asarray(weights[f"asrc{li+1}"], np.float32)
        a_d = np.asarray(weights[f"adst{li+1}"], np.float32)
        b = np.asarray(weights[f"b{li+1}"], np.float32)
        wext = np.concatenate([W, (W @ a_s)[:, None], (W @ a_d)[:, None]], axis=1)
        ins_shared[f"wext{li}"] = wext.astype(NPBF16)
        ins_shared[f"bias{li}"] = b[None, :].astype(np.float32)
    for name in TAIL_SHAPES:
        ins_shared[name + "_w"] = np.asarray(weights[name + "_w"], np.float32)
        ins_shared[name + "_b"] = np.asarray(weights[name + "_b"], np.float32)[:, None]
    ins_shared["recip_cnt"] = pp["recip_cnt"][None, :]
    ins_shared["ones_row"] = np.ones((1, P), np.float32)

    in_maps = []
    for c in range(NCORES):
        m = dict(ins_shared)
        m["xcore"] = pp["xcore"][c].astype(NPBF16)
        m["idx16"] = pp["idx16"][c]
        m["dstloc"] = pp["dstloc"][c]
        m["padmaskT"] = pp["padT"][c]
        m["padoffT"] = (-3.0e38 * (1.0 - pp["padT"][c])).astype(np.float32)
        m["slot_mask"] = pp["slot_mask"][c]
        in_maps.append(m)
    return in_maps


def build_kernel(pp):
    layer_dims = pp["layer_dims"]
    T, TOWN, SH, NTT, VIEWB = pp["T"], pp["TOWN"], pp["SH"], pp["NTT"], pp["VIEWB"]
    K_lo, K_hi, nchunks = pp["K_lo"], pp["K_hi"], pp["nchunks"]
    G = pp["G"]
    F_IN = pp["F_IN"]
    gsrc = pp["gsrc"]
    NL = len(layer_dims)
    d3out = layer_dims[-1][1]          # 256
    NH3 = d3out // P                   # 2
    ROWS = [row_elems(do) for (_, do) in layer_dims]

    nc = bacc.Bacc("TRN2", num_devices=NCORES, num_swdge_queues=4)
    dp = nc.declare_dram_parameter

    t_xc = dp("xcore", [F_IN, TOWN * P], BF16, isOutput=False)
    t_idx = dp("idx16", [P, nchunks * 8], I16, isOutput=False)
    t_dstloc = dp("dstloc", [P, nchunks], F32, isOutput=False)
    t_ones = dp("ones_row", [1, P], F32, isOutput=False)
    t_wext, t_bias = [], []
    for li, (di, do) in enumerate(layer_dims):
        t_wext.append(dp(f"wext{li}", [di, do + 2], BF16, isOutput=False))
        t_bias.append(dp(f"bias{li}", [1, do], F32, isOutput=False))
    t_padm = dp("padmaskT", [P, TOWN], F32, isOutput=False)
    t_pado = dp("padoffT", [P, TOWN], F32, isOutput=False)
    t_slotm = dp("slot_mask", [MAXSLOT, TOWN], F32, isOutput=False)
    t_rcnt = dp("recip_cnt", [1, G], F32, isOutput=False)
    t_tail = {}
    for name, (a, b) in TAIL_SHAPES.items():
        t_tail[name + "_w"] = dp(name + "_w", [a, b], F32, isOutput=False)
        t_tail[name + "_b"] = dp(name + "_b", [b, 1], F32, isOutput=False)
    t_out = dp("z_out", [G, 128], F32, isOutput=True)

    stage = [nc.dram_tensor(f"stage{li}", [SH, ROWS[li]], BF16)
             for li in range(NL)]
    s2dram = [nc.dram_tensor(f"s2d{li}", [TOWN, P], F32) for li in range(NL)]
    tab = [nc.dram_tensor(f"tab{li}", [NTT, ROWS[li]], BF16, addr_space="Shared")
           for li in range(NL)]
    POOLW = 2 * NH3 * MAXSLOT
    pool_shard = nc.dram_tensor("pool_shard", [P, POOLW], F32)
    pool_all = nc.dram_tensor("pool_all", [NCORES * P, POOLW], F32,
                              addr_space="Shared")

    RG = [list(range(NCORES))]
    Exp = mybir.ActivationFunctionType.Exp
    Relu = mybir.ActivationFunctionType.Relu
    Sig = mybir.ActivationFunctionType.Sigmoid
    EQ = mybir.AluOpType.is_equal
    MUL = mybir.AluOpType.mult
    ADD = mybir.AluOpType.add
    MAX = mybir.AluOpType.max

    with tile.TileContext(nc) as tc:
        with (
            tc.tile_pool(name="const", bufs=1) as cb,
            tc.tile_pool(name="sbuf", bufs=3) as sb,
            tc.tile_pool(name="gat", bufs=3) as sg,
            tc.tile_pool(name="msk", bufs=4) as sm,
            tc.tile_pool(name="psA", bufs=2, space="PSUM") as psA,
            tc.tile_pool(name="psN", bufs=2, space="PSUM") as psN,
            tc.tile_pool(name="psS", bufs=3, space="PSUM") as psS,
            tc.tile_pool(name="psP", bufs=1, space="PSUM") as psP,
        ):
            # ---------------- constants ----------------
            iota_i = cb.tile([P, P], I32)
            nc.gpsimd.iota(iota_i[:], pattern=[[1, P]], base=0, channel_multiplier=0)
            iota_bf = cb.tile([P, P], BF16)
            nc.vector.tensor_copy(out=iota_bf[:], in_=iota_i[:])
            ident = cb.tile([P, P], F32)
            make_identity(nc, ident[:])
            ones_row = cb.tile([1, P], F32)
            nc.sync.dma_start(out=ones_row[:], in_=t_ones[:])

            idx_sb = cb.tile([P, nchunks * 8], I16)
            nc.sync.dma_start(out=idx_sb[:], in_=t_idx[:])
            dstloc_sb = cb.tile([P, nchunks], F32)
            nc.sync.dma_start(out=dstloc_sb[:], in_=t_dstloc[:])
            padm_sb = cb.tile([P, TOWN], F32)
            nc.sync.dma_start(out=padm_sb[:], in_=t_padm[:])
            pado_sb = cb.tile([P, TOWN], F32)
            nc.sync.dma_start(out=pado_sb[:], in_=t_pado[:])

            wext_sb, bias_bc = [], []
            for li, (di, do) in enumerate(layer_dims):
                w = cb.tile([di, do + 2], BF16, tag=f"wext{li}")
                nc.sync.dma_start(out=w[:], in_=t_wext[li][:])
                wext_sb.append(w)
                brow = cb.tile([1, do], F32, tag=f"brow{li}")
                nc.sync.dma_start(out=brow[:], in_=t_bias[li][:])
                bps = psS.tile([P, do], F32, space="PSUM", tag="ps")
                nc.tensor.matmul(out=bps[:], lhsT=ones_row[:], rhs=brow[:],
                                 start=True, stop=True)
                bb = cb.tile([P, do], F32, tag=f"biasbc{li}")
                nc.vector.tensor_copy(out=bb[:], in_=bps[:])
                bias_bc.append(bb)

            # pad-row template per layer: zeros + NEG_BIG at s1 column
            def write_pad_tile(li, do):
                pad_t = sb.tile([P, ROWS[li]], BF16, tag="padtile")
                nc.vector.memset(pad_t[:], 0.0)
                nc.vector.memset(pad_t[:, do + 1:do + 2], NEG_BIG)
                nc.sync.dma_start(out=stage[li][TOWN * P:(TOWN + 1) * P, :],
                                  in_=pad_t[:])

            # s2 per layer, flat [1, TOWN*P] (single partition for matmul rhs)
            s2flat = [None] * NL

            def finish_s2(li, s2loc):
                tp = psS.tile([TOWN, P], F32, space="PSUM", tag="ps")
                nc.tensor.transpose(out=tp[:], in_=s2loc[:, 0:TOWN], identity=ident[:])
                so = sb.tile([TOWN, P], F32, tag="s2own")
                nc.vector.tensor_copy(out=so[:], in_=tp[:])
                nc.sync.dma_start(out=s2dram[li][:, :], in_=so[:])
                sf = sb.tile([1, TOWN * P], F32, tag="s2flat")
                nc.sync.dma_start(
                    out=sf[:],
                    in_=s2dram[li][:, :].rearrange("a b -> (a b)").unsqueeze(0))
                s2flat[li] = sf

            # write projected rows (h' | 1 | s1') into the stage batch, keep s2'
            def stage_row(li, hp, do, s, s2loc):
                b = s % WBATCH
                if b == 0:
                    stage_row.buf = sb.tile([P, WBATCH, ROWS[li]], BF16, tag="stgbuf")
                buf = stage_row.buf
                nc.vector.tensor_copy(out=buf[:, b, 0:do], in_=hp[:, 0:do])
                nc.vector.memset(buf[:, b, do:do + 1], 1.0)
                nc.vector.tensor_copy(out=buf[:, b, do + 1:do + 2],
                                      in_=hp[:, do:do + 1])
                nc.vector.tensor_copy(out=s2loc[:, s:s + 1], in_=hp[:, do + 1:do + 2])
                if b == WBATCH - 1 or s == TOWN - 1:
                    s0 = s - b
                    nb = b + 1
                    nc.sync.dma_start(
                        out=stage[li][s0 * P:(s0 + nb) * P, :].rearrange(
                            "(c p) f -> p c f", p=P),
                        in_=buf[:, 0:nb, :])

            # ---------------- L1 phase A (own tiles only) ----------------
            do0 = layer_dims[0][1]
            s2loc0 = sb.tile([P, TOWN], F32, tag="s2loc")
            with tc.tile_pool(name="xc", bufs=1) as xcp:
                xc_sb = xcp.tile([F_IN, TOWN * P], BF16)
                nc.sync.dma_start(out=xc_sb[:], in_=t_xc[:])
                for s in range(TOWN):
                    hp = psA.tile([P, do0 + 2], F32, space="PSUM", tag="hp")
                    nc.tensor.matmul(out=hp[:], lhsT=xc_sb[:, s * P:(s + 1) * P],
                                     rhs=wext_sb[0][:], start=True, stop=True)
                    stage_row(0, hp, do0, s, s2loc0)
            write_pad_tile(0, do0)
            finish_s2(0, s2loc0)
            # NOTE: consumers of a collective's DRAM output are NOT auto-
            # tracked by Tile's dep pass (verified: gather dep closure lacks
            # the cc) -- add explicit edges below via add_dep_helper.
            tab_ag = [None] * NL
            tab_ag[0] = nc.gpsimd.collective_compute(
                "AllGather", mybir.AluOpType.bypass, replica_groups=RG,
                ins=[stage[0][:, :]], outs=[tab[0][:, :]])

            # ---------------- layers ----------------
            gq = 0
            ch = 0
            rmax_stage = cb.tile([P, NH3, TOWN], F32)
            rsum_ps = psP.tile([P, NH3, TOWN], F32, space="PSUM", tag="psum_pool")
            NLAYERS = int(os.environ.get("GAT_LAYERS", NL))
            for li in range(NLAYERS):
                di, do = layer_dims[li]
                ROW = ROWS[li]
                last = li == NL - 1
                if not last:
                    don = layer_dims[li + 1][1]
                    s2loc_n = sb.tile([P, TOWN], F32, tag="s2loc")
                ch = 0     # same gather indices every layer
                for s in range(TOWN):
                    # S2B: s2 of this tile's nodes broadcast down partitions
                    sps = psS.tile([P, P], F32, space="PSUM", tag="ps")
                    nc.tensor.matmul(out=sps[:], lhsT=ones_row[:],
                                     rhs=s2flat[li][:, s * P:(s + 1) * P],
                                     start=True, stop=True)
                    S2B = sm.tile([P, P], BF16, tag="S2B")
                    nc.vector.tensor_copy(out=S2B[:], in_=sps[:])

                    num = psN.tile([P, do + 1], F32, space="PSUM", tag="num")
                    ktot = int(K_lo[s] + K_hi[s])
                    done = 0
                    for half in (0, 1):
                        K = int(K_lo[s]) if half == 0 else int(K_hi[s])
                        view = tab[li][0:32768, :] if half == 0 \
                            else tab[li][VIEWB:NTT, :]
                        for c0 in range(0, K, CALL_CHUNKS):
                            k = min(CALL_CHUNKS, K - c0)
                            G_t = sg.tile([P, k, ROW], BF16, tag="G")
                            g_inst = nc.gpsimd.dma_gather(
                                G_t[:], view, idx_sb[:, ch * 8:(ch + k) * 8],
                                k * P, k * P, ROW, queue_num=gq % 4)
                            add_dep_helper(g_inst.ins, tab_ag[li].ins, True,
                                           "gather waits for table AllGather")
                            gq += 1
                            s2e = sm.tile([P, k], BF16, tag="s2e")
                            for j in range(k):
                                scr = sm.tile([P, P], BF16, tag="scr")
                                nc.vector.scalar_tensor_tensor(
                                    out=scr[:], in0=iota_bf[:],
                                    scalar=dstloc_sb[:, ch + j:ch + j + 1],
                                    in1=S2B[:], op0=EQ, op1=MUL,
                                    accum_out=s2e[:, j:j + 1])
                            raw = sm.tile([P, k], BF16, tag="raw")
                            nc.vector.tensor_add(
                                out=raw[:], in0=G_t[:, :, do + 1], in1=s2e[:])
                            r2 = sm.tile([P, k], BF16, tag="r2")
                            nc.vector.tensor_scalar_mul(r2[:], raw[:], NEG_SLOPE)
                            lr = sm.tile([P, k], BF16, tag="lr")
                            nc.vector.tensor_max(out=lr[:], in0=raw[:], in1=r2[:])
                            ex = sm.tile([P, k], F32, tag="ex")
                            nc.scalar.activation(ex[:], lr[:], Exp)
                            for j in range(k):
                                sel = sm.tile([P, P], BF16, tag="sel")
                                nc.vector.tensor_scalar(
                                    out=sel[:], in0=iota_bf[:],
                                    scalar1=dstloc_sb[:, ch + j:ch + j + 1],
                                    scalar2=ex[:, j:j + 1],
                                    op0=EQ, op1=MUL)
                                nc.tensor.matmul(out=num[:], lhsT=sel[:],
                                                 rhs=G_t[:, j, 0:do + 1],
                                                 start=(done == 0),
                                                 stop=(done == ktot - 1))
                                done += 1
                                ch += 1
                    # ---- epilogue ----
                    den = sm.tile([P, 1], F32, tag="den")
                    nc.vector.tensor_scalar(out=den[:], in0=num[:, do:do + 1],
                                            scalar1=1.0e-30, scalar2=None, op0=MAX)
                    rden = sm.tile([P, 1], F32, tag="rden")
                    nc.vector.reciprocal(out=rden[:], in_=den[:])
                    outb = sb.tile([P, do], F32, tag="outb")
                    if last:
                        bm = sb.tile([P, do], F32, tag="bm")
                        nc.vector.tensor_scalar(out=bm[:], in0=bias_bc[li][:],
                                                scalar1=padm_sb[:, s:s + 1],
                                                scalar2=None, op0=MUL)
                        badd = bm
                    else:
                        badd = bias_bc[li]
                    nc.vector.scalar_tensor_tensor(
                        out=outb[:], in0=num[:, 0:do], scalar=rden[:, 0:1],
                        in1=badd[:], op0=MUL, op1=ADD)
                    if not last:
                        # next-layer projection -> stage rows
                        tp = psS.tile([do, P], F32, space="PSUM", tag="ps")
                        nc.tensor.transpose(out=tp[:], in_=outb[:, 0:do],
                                            identity=ident[:])
                        oT = sb.tile([do, P], BF16, tag="oT")
                        nc.vector.tensor_copy(out=oT[:], in_=tp[:])
                        hp2 = psA.tile([P, don + 2], F32, space="PSUM", tag="hp")
                        nc.tensor.matmul(out=hp2[:], lhsT=oT[:],
                                         rhs=wext_sb[li + 1][:],
                                         start=True, stop=True)
                        stage_row(li + 1, hp2, don, s, s2loc_n)
                    else:
                        # pooling: sum via N=1 matmuls, max via pad offsets
                        for hf in range(NH3):
                            nc.tensor.matmul(
                                out=rsum_ps[:, hf, s:s + 1],
                                lhsT=outb[:, hf * P:(hf + 1) * P],
                                rhs=padm_sb[:, s:s + 1], start=True, stop=True)
                            fm = sb.tile([P, P], F32, tag="fm")
                            nc.vector.tensor_scalar(
                                out=fm[:], in0=outb[:, hf * P:(hf + 1) * P],
                                scalar1=pado_sb[:, s:s + 1], scalar2=None, op0=ADD)
                            fmt = psS.tile([P, P], F32, space="PSUM", tag="ps")
                            nc.tensor.transpose(out=fmt[:], in_=fm[:],
                                                identity=ident[:])
                            nc.vector.tensor_reduce(
                                out=rmax_stage[:, hf, s:s + 1], in_=fmt[:],
                                axis=mybir.AxisListType.X, op=MAX)
                if not last:
                    write_pad_tile(li + 1, don)
                    finish_s2(li + 1, s2loc_n)
                    tab_ag[li + 1] = nc.gpsimd.collective_compute(
                        "AllGather", mybir.AluOpType.bypass, replica_groups=RG,
                        ins=[stage[li + 1][:, :]], outs=[tab[li + 1][:, :]])

            # ---------------- pooling combine + tail ----------------
            if NLAYERS < NL:
                zstub = sb.tile([G, 128], F32, tag="zstub")
                nc.vector.memset(zstub[:], 0.0)
                nc.sync.dma_start(out=t_out[:, :], in_=zstub[:])
            else:
                rsum_stage = cb.tile([P, NH3, TOWN], F32)
                nc.vector.tensor_copy(out=rsum_stage[:], in_=rsum_ps[:])

                pool_loc = cb.tile([P, 2, NH3, MAXSLOT], F32)
                for sl in range(MAXSLOT):
                    smrow = sb.tile([1, TOWN], F32, tag="smrow")
                    nc.sync.dma_start(out=smrow[:], in_=t_slotm[sl:sl + 1, :])
                    smb_ps = psS.tile([P, TOWN], F32, space="PSUM", tag="ps")
                    nc.tensor.matmul(out=smb_ps[:], lhsT=ones_row[:],
                                     rhs=smrow[:], start=True, stop=True)
                    smb = sb.tile([P, TOWN], F32, tag="smb")
                    nc.vector.tensor_copy(out=smb[:], in_=smb_ps[:])
                    # 0 where mask=1, -3e38 where mask=0
                    sob = sb.tile([P, TOWN], F32, tag="sob")
                    nc.vector.tensor_scalar(out=sob[:], in0=smb[:],
                                            scalar1=3.0e38, scalar2=-3.0e38,
                                            op0=mybir.AluOpType.mult,
                                            op1=mybir.AluOpType.add)
                    for hf in range(NH3):
                        mz = sb.tile([P, TOWN], F32, tag="mz")
                        nc.vector.tensor_mul(out=mz[:], in0=rsum_stage[:, hf, :],
                                             in1=smb[:])
                        nc.vector.tensor_reduce(
                            out=pool_loc[:, 1, hf, sl:sl + 1], in_=mz[:],
                            axis=mybir.AxisListType.X, op=ADD)
                        mm2 = sb.tile([P, TOWN], F32, tag="mm2")
                        nc.vector.tensor_add(out=mm2[:], in0=rmax_stage[:, hf, :],
                                             in1=sob[:])
                        nc.vector.tensor_reduce(
                            out=pool_loc[:, 0, hf, sl:sl + 1], in_=mm2[:],
                            axis=mybir.AxisListType.X, op=MAX)
                pl_sb = sb.tile([P, POOLW], F32, tag="plf")
                nc.vector.tensor_copy(
                    out=pl_sb[:], in_=pool_loc[:].rearrange("p a b c -> p (a b c)"))
                nc.sync.dma_start(out=pool_shard[:, :], in_=pl_sb[:])
                pool_ag = nc.gpsimd.collective_compute(
                    "AllGather", mybir.AluOpType.bypass, replica_groups=RG,
                    ins=[pool_shard[:, :]], outs=[pool_all[:, :]])
                pa = cb.tile([P, NCORES, POOLW], F32)
                pa_ld = nc.sync.dma_start(
                    out=pa[:], in_=pool_all[:, :].rearrange("(c p) w -> p c w", p=P))
                add_dep_helper(pa_ld.ins, pool_ag.ins, True,
                               "pool read waits for pool AllGather")

                x1T = cb.tile([P, NH3, G], F32)
                x2T = cb.tile([P, NH3, G], F32)
                for g in range(G):
                    (cA, sA), (cB, sB) = gsrc[g]
                    for hf in range(NH3):
                        iA = 0 * NH3 * MAXSLOT + hf * MAXSLOT + sA
                        iB = 0 * NH3 * MAXSLOT + hf * MAXSLOT + sB
                        nc.vector.tensor_max(out=x1T[:, hf, g:g + 1],
                                             in0=pa[:, cA, iA:iA + 1],
                                             in1=pa[:, cB, iB:iB + 1])
                        jA = 1 * NH3 * MAXSLOT + hf * MAXSLOT + sA
                        jB = 1 * NH3 * MAXSLOT + hf * MAXSLOT + sB
                        if (cA, sA) == (cB, sB):
                            nc.vector.tensor_copy(out=x2T[:, hf, g:g + 1],
                                                  in_=pa[:, cA, jA:jA + 1])
                        else:
                            nc.vector.tensor_add(out=x2T[:, hf, g:g + 1],
                                                 in0=pa[:, cA, jA:jA + 1],
                                                 in1=pa[:, cB, jB:jB + 1])
                rc_row = cb.tile([1, G], F32)
                nc.sync.dma_start(out=rc_row[:], in_=t_rcnt[:])
                rcb_ps = psS.tile([P, G], F32, space="PSUM", tag="ps")
                nc.tensor.matmul(out=rcb_ps[:], lhsT=ones_row[:], rhs=rc_row[:],
                                 start=True, stop=True)
                rcb = cb.tile([P, G], F32)
                nc.vector.tensor_copy(out=rcb[:], in_=rcb_ps[:])
                x3T = cb.tile([P, NH3, G], F32)
                for hf in range(NH3):
                    nc.vector.tensor_mul(out=x3T[:, hf, :], in0=x2T[:, hf, :],
                                         in1=rcb[:])

                # ---------------- dense tail ([feat, G] layout) ----------------
                def load_w(name, r0, rows, c0, cols):
                    wt = sb.tile([rows, cols], F32, tag="tw")
                    nc.sync.dma_start(
                        out=wt[:], in_=t_tail[name + "_w"][r0:r0 + rows, c0:c0 + cols])
                    return wt

                def load_b(name, c0, rows):
                    bt = sb.tile([rows, 1], F32, tag="tb")
                    nc.sync.dma_start(out=bt[:], in_=t_tail[name + "_b"][c0:c0 + rows, :])
                    return bt

                def dense_T(name, zparts, din, dout, act):
                    outs = []
                    nko = cdiv(dout, P)
                    nki = cdiv(din, P)
                    for ho in range(nko):
                        wo = min(P, dout - ho * P)
                        pso = psN.tile([P, G], F32, space="PSUM", tag="num")
                        for hi in range(nki):
                            wi = min(P, din - hi * P)
                            wt = load_w(name, hi * P, wi, ho * P, wo)
                            nc.tensor.matmul(out=pso[:wo, :], lhsT=wt[:],
                                             rhs=zparts[hi][:wi, :],
                                             start=(hi == 0), stop=(hi == nki - 1))
                        bt = load_b(name, ho * P, wo)
                        ot = sb.tile([P, G], F32, tag=f"t_{name}_{ho}")
                        nc.scalar.activation(ot[:wo, :], pso[:wo, :], act,
                                             bias=bt[:, 0:1])
                        outs.append(ot)
                    return outs

                Copy = mybir.ActivationFunctionType.Copy
                z0 = [x1T[:, 0, :], x1T[:, 1, :], x2T[:, 0, :], x2T[:, 1, :]]
                z1 = dense_T("d1", z0, 2 * d3out, 256, Relu)
                z2 = dense_T("d2", z1, 256, 128, Relu)
                z3 = dense_T("d3", z2, 128, 64, Relu)
                gate = dense_T("mean", [x3T[:, h, :] for h in range(NH3)],
                               d3out, 64, Sig)
                z4 = sb.tile([P, G], F32, tag="z4")
                nc.vector.tensor_mul(out=z4[:64, :], in0=z3[0][:64, :],
                                     in1=gate[0][:64, :])
                z5 = dense_T("d4", [z4], 64, 64, Relu)
                z6 = dense_T("d5", z5, 64, 128, Relu)
                z7 = dense_T("d6", z6, 128, 256, Relu)
                pso = psN.tile([G, P], F32, space="PSUM", tag="num")
                for hi in range(2):
                    wt = load_w("d7", hi * P, P, 0, P)
                    nc.tensor.matmul(out=pso[:, :], lhsT=z7[hi][:, :], rhs=wt[:],
                                     start=(hi == 0), stop=(hi == 1))
                fb_row = sb.tile([1, P], F32, tag="fbrow")
                nc.sync.dma_start(out=fb_row[:],
                                  in_=t_tail["d7_b"][:, :].rearrange("a b -> b a"))
                onesG = cb.tile([1, G], F32)
                nc.vector.memset(onesG[:], 1.0)
                fbb_ps = psS.tile([G, P], F32, space="PSUM", tag="ps")
                nc.tensor.matmul(out=fbb_ps[:, :], lhsT=onesG[:], rhs=fb_row[:],
                                 start=True, stop=True)
                fbb_sb = sb.tile([G, P], F32, tag="fbbsb")
                nc.vector.tensor_copy(out=fbb_sb[:], in_=fbb_ps[:])
                zf = sb.tile([G, P], F32, tag="zf")
                nc.vector.tensor_add(out=zf[:], in0=pso[:], in1=fbb_sb[:])
                nc.sync.dma_start(out=t_out[:, :], in_=zf[:])

    nc.compile()
    return nc


# ======================= kernel entry =======================
G_GRAPHS = 64
LAYER_DIMS = [(128, 64), (64, 128), (128, 256)]
LAST_EXEC_NS = None
LAST_RES = None

_cache = {}


def kernel(x, edge_index, batch, **weights):
    global LAST_EXEC_NS, LAST_RES
    from concourse.bass_utils import run_bass_kernel_spmd
    x = np.asarray(x, np.float32)
    edge_index = np.asarray(edge_index)
    batch = np.asarray(batch)

    pp = preprocess(x, edge_index, batch, G_GRAPHS, LAYER_DIMS)
    in_maps = make_inputs(pp, weights)
    key = (pp["T"], pp["nchunks"], tuple(pp["K_lo"]), tuple(pp["K_hi"]))
    if key not in _cache:
        _cache[key] = build_kernel(pp)
    nc = _cache[key]
    trace = bool(os.environ.get("GAT_TRACE"))
    kw = {}
    if trace and os.environ.get("GAT_TRACE_DIR"):
        os.makedirs(os.environ["GAT_TRACE_DIR"], exist_ok=True)
        kw["tmpdir"] = os.environ["GAT_TRACE_DIR"]
    res = run_bass_kernel_spmd(nc, in_maps, list(range(NCORES)), trace=trace, **kw)
    LAST_EXEC_NS = res.exec_time_ns
    LAST_RES = res
    return res.results[0]["z_out"].astype(np.float32)
